# revision 1
# baseline (speedup 1.0000x reference)
"""AttentionBlock Trainium2 kernel (nn_AttentionBlock dense_transformer).

Sharding: data-parallel over batch B=8 across 8 NeuronCores (1 image/core).
Per-core pipeline:
  - GroupNorm(32 groups) over x [512, 1024]
  - qkv / encoder_kv projections (bf16 matmuls, fp32 PSUM accumulate)
      q,k in [c, t] layout (orientation A), v/ev transposed [s, c] (orientation B)
  - attention: S^T = k^T q in [s, t] layout; softmax axis = partitions.
      Max-subtraction is skipped (logits are O(6) by construction: normalized
      activations x unit-variance weights, scale folded on host).
      exp on ScalarE; A = sum_s P v via col-packed matmuls; denominator D via
      ones-lhsT matmuls col-packed 4-way; P/D applied during PSUM->SBUF copy.
  - proj + residual add
All matmul inputs bf16 (fp32 accumulation); end-to-end error vs fp32 reference
measured ~5e-4 relative.
"""

import numpy as np
import ml_dtypes

B, C, H, W = 8, 512, 32, 32
L = H * W                      # 1024
NH = 8
CH = C // NH                   # 64 per head
G = 32                         # groupnorm groups
GS = C // G                    # 16 channels per group
ENC_C, ENC_L = 768, 77
EPS = 1e-5
S_TOT = ENC_L + L              # 1101
SCALE = 1.0 / np.sqrt(np.sqrt(CH))
N_CORES = 8

# s-chunks of the key/value axis: enc block (77) then 8 x 128 self blocks
S_CHUNKS = [(0, ENC_L)] + [(ENC_L + 128 * i, 128) for i in range(8)]

BF16 = ml_dtypes.bfloat16


def _build_bass(debug=False):
    import concourse.bass as bass
    import concourse.mybir as mybir
    import concourse.tile as tile
    from concourse import bacc

    f32 = mybir.dt.float32
    bf = mybir.dt.bfloat16
    AF = mybir.ActivationFunctionType
    OP = mybir.AluOpType

    nc = bacc.Bacc()

    # ---- DRAM I/O ----
    x_d = nc.dram_tensor("x", [C, L], f32, kind="ExternalInput")
    enc_d = nc.dram_tensor("enc", [ENC_C, ENC_L], bf, kind="ExternalInput")
    wqk_d = nc.dram_tensor("wqk", [C, 1024], bf, kind="ExternalInput")
    wv_d = nc.dram_tensor("wv", [C, 512], bf, kind="ExternalInput")
    wek_d = nc.dram_tensor("wek", [ENC_C, 512], bf, kind="ExternalInput")
    wev_d = nc.dram_tensor("wev", [ENC_C, 512], bf, kind="ExternalInput")
    wp_d = nc.dram_tensor("wp", [C, C], bf, kind="ExternalInput")
    bqk_d = nc.dram_tensor("bqk", [128, 8], f32, kind="ExternalInput")
    bek_d = nc.dram_tensor("bek", [128, 4], f32, kind="ExternalInput")
    bv_d = nc.dram_tensor("bv", [1, 512], bf, kind="ExternalInput")
    bev_d = nc.dram_tensor("bev", [1, 512], bf, kind="ExternalInput")
    bp_d = nc.dram_tensor("bp", [128, 4], f32, kind="ExternalInput")
    gnw_d = nc.dram_tensor("gnw", [128, 4], f32, kind="ExternalInput")
    gnb_d = nc.dram_tensor("gnb", [128, 4], f32, kind="ExternalInput")
    emat_d = nc.dram_tensor("emat", [128, 8], bf, kind="ExternalInput")
    etmat_d = nc.dram_tensor("etmat", [8, 128], bf, kind="ExternalInput")
    out_d = nc.dram_tensor("out", [C, L], f32, kind="ExternalOutput")

    with tile.TileContext(nc) as tc:
        with tc.tile_pool(name="wpool", bufs=1) as wpool, \
             tc.tile_pool(name="data", bufs=1) as data, \
             tc.tile_pool(name="small", bufs=1) as small, \
             tc.tile_pool(name="pts", bufs=6) as pts, \
             tc.tile_pool(name="ddr", bufs=2, space="DRAM") as ddr_pool, \
             tc.tile_pool(name="mm_ps", bufs=2, space="PSUM") as mm_ps, \
             tc.tile_pool(name="st_ps", bufs=2, space="PSUM") as st_ps, \
             tc.tile_pool(name="av_ps", bufs=2, space="PSUM") as av_ps:

            # ---------------- loads, in consumption order ----------------
            xt = [data.tile([128, 1024], f32, name=f"xt{k}") for k in range(4)]
            for k in range(4):
                eng = nc.sync if k % 2 == 0 else nc.gpsimd
                eng.dma_start(out=xt[k], in_=x_d[128 * k:128 * (k + 1), :])
            enct = [data.tile([128, ENC_L], bf, name=f"enct{k}") for k in range(6)]
            for k in range(6):
                nc.sync.dma_start(out=enct[k], in_=enc_d[128 * k:128 * (k + 1), :])
            wek = [wpool.tile([128, 512], bf, name=f"wek{k}") for k in range(6)]
            wev = [wpool.tile([128, 512], bf, name=f"wev{k}") for k in range(6)]
            for k in range(6):
                nc.sync.dma_start(out=wek[k], in_=wek_d[128 * k:128 * (k + 1), :])
                nc.gpsimd.dma_start(out=wev[k], in_=wev_d[128 * k:128 * (k + 1), :])
            wqk = [wpool.tile([128, 1024], bf, name=f"wqk{k}") for k in range(4)]
            for k in range(4):
                nc.gpsimd.dma_start(out=wqk[k], in_=wqk_d[128 * k:128 * (k + 1), :])
            wv = [wpool.tile([128, 512], bf, name=f"wv{k}") for k in range(4)]
            for k in range(4):
                nc.gpsimd.dma_start(out=wv[k], in_=wv_d[128 * k:128 * (k + 1), :])
            wp = [wpool.tile([128, 512], bf, name=f"wp{k}") for k in range(4)]
            for k in range(4):
                nc.gpsimd.dma_start(out=wp[k], in_=wp_d[128 * k:128 * (k + 1), :])
            bqk = wpool.tile([128, 8], f32)
            nc.sync.dma_start(out=bqk, in_=bqk_d[:, :])
            bek = wpool.tile([128, 4], f32)
            nc.sync.dma_start(out=bek, in_=bek_d[:, :])
            bv = wpool.tile([1, 512], bf)
            nc.sync.dma_start(out=bv, in_=bv_d[:, :])
            bev = wpool.tile([1, 512], bf)
            nc.sync.dma_start(out=bev, in_=bev_d[:, :])
            bp = wpool.tile([128, 4], f32)
            nc.sync.dma_start(out=bp, in_=bp_d[:, :])
            gnw = wpool.tile([128, 4], f32)
            nc.sync.dma_start(out=gnw, in_=gnw_d[:, :])
            gnb = wpool.tile([128, 4], f32)
            nc.sync.dma_start(out=gnb, in_=gnb_d[:, :])
            emat = wpool.tile([128, 8], bf)
            nc.sync.dma_start(out=emat, in_=emat_d[:, :])
            etmat = wpool.tile([8, 128], bf)
            nc.sync.dma_start(out=etmat, in_=etmat_d[:, :])

            ones_col = wpool.tile([128, 1], bf)   # lhsT for denominator matmuls
            nc.vector.memset(ones_col, 1.0)
            ones_row = wpool.tile([1, 128], bf)   # lhsT for K=1 bias matmuls
            nc.vector.memset(ones_row, 1.0)

            # ---------------- encoder kv (small, first) ----------------
            ek = [data.tile([128, ENC_L], bf, name=f"ek{p}") for p in range(4)]
            evT = data.tile([ENC_L, 512], bf)
            with nc.named_scope("ekv"):
                for p in range(4):
                    ps = mm_ps.tile([128, ENC_L], f32, name="ek_ps", tag="mm")
                    for k in range(6):
                        nc.tensor.matmul(
                            ps, wek[k][:, 128 * p:128 * (p + 1)], enct[k],
                            start=(k == 0), stop=(k == 5))
                    nc.vector.tensor_scalar_add(out=ek[p], in0=ps, scalar1=bek[:, p:p + 1])
                ps = mm_ps.tile([ENC_L, 512], f32, name="ev_ps", tag="mm")
                for k in range(6):
                    nc.tensor.matmul(ps, enct[k], wev[k], start=(k == 0), stop=False)
                nc.tensor.matmul(ps, ones_row[:, 0:ENC_L], bev, start=False, stop=True)
                nc.vector.tensor_copy(out=evT, in_=ps)

            # ---------------- GroupNorm ----------------
            with nc.named_scope("gn"):
                stats = small.tile([128, 8], f32)
                for k in range(4):
                    nc.vector.reduce_sum(stats[:, k:k + 1], xt[k], axis=mybir.AxisListType.X)
                for k in range(4):
                    xsq = small.tile([128, 1024], f32, name="xsq", tag="xsq", bufs=2)
                    nc.scalar.activation(out=xsq, in_=xt[k], func=AF.Square,
                                         accum_out=stats[:, 4 + k:5 + k])
                stats_bf = small.tile([128, 8], bf)
                nc.vector.tensor_copy(out=stats_bf, in_=stats)
                g8_ps = mm_ps.tile([8, 8], f32, name="g8", tag="mm")
                nc.tensor.matmul(g8_ps, emat, stats_bf, start=True, stop=True)
                musg = small.tile([8, 8], f32)   # cols 0:4 mean, 4:8 later rstd
                inv_n = 1.0 / (GS * L)
                nc.vector.tensor_scalar_mul(out=musg, in0=g8_ps, scalar1=inv_n)
                var8 = small.tile([8, 4], f32)
                nc.vector.tensor_mul(out=var8, in0=musg[:, 0:4], in1=musg[:, 0:4])
                nc.vector.tensor_sub(out=var8, in0=musg[:, 4:8], in1=var8)
                epst = small.tile([8, 1], f32)
                nc.vector.memset(epst, EPS)
                lnv = small.tile([8, 4], f32)
                nc.scalar.activation(out=lnv, in_=var8, func=AF.Ln, bias=epst, scale=1.0)
                nc.scalar.activation(out=musg[:, 4:8], in_=lnv, func=AF.Exp, scale=-0.5)
                musg_bf = small.tile([8, 8], bf)
                nc.vector.tensor_copy(out=musg_bf, in_=musg)
                exp_ps = mm_ps.tile([128, 8], f32, name="exp_ps", tag="mm")
                nc.tensor.matmul(exp_ps, etmat, musg_bf, start=True, stop=True)
                aff_a = small.tile([128, 4], f32)
                nc.vector.tensor_mul(out=aff_a, in0=gnw, in1=exp_ps[:, 4:8])
                aff_b = small.tile([128, 4], f32)
                nc.vector.tensor_mul(out=aff_b, in0=exp_ps[:, 0:4], in1=aff_a)
                nc.vector.tensor_sub(out=aff_b, in0=gnb, in1=aff_b)
                hn = [data.tile([128, 1024], bf, name=f"hn{k}") for k in range(4)]
                for k in range(4):
                    eng = nc.vector if k % 2 == 0 else nc.gpsimd
                    eng.tensor_scalar(
                        out=hn[k], in0=xt[k], scalar1=aff_a[:, k:k + 1],
                        scalar2=aff_b[:, k:k + 1], op0=OP.mult, op1=OP.add)

            # ---------------- projections + attention, interleaved ----------------
            qk = [data.tile([128, 1024], bf, name=f"qk{m}") for m in range(8)]
            vT = [data.tile([128, 512], bf, name=f"vT{m}") for m in range(8)]
            a_sb = [data.tile([128, 1024], bf, name=f"a_sb{p}") for p in range(4)]

            def emit_qk(m):
                for n in range(2):
                    ps = mm_ps.tile([128, 512], f32, name="qkv_ps", tag="mm")
                    for k in range(4):
                        nc.tensor.matmul(
                            ps, wqk[k][:, 128 * m:128 * (m + 1)],
                            hn[k][:, 512 * n:512 * (n + 1)],
                            start=(k == 0), stop=(k == 3))
                    nc.vector.tensor_scalar_add(
                        out=qk[m][:, 512 * n:512 * (n + 1)], in0=ps,
                        scalar1=bqk[:, m:m + 1])

            def emit_vT(m):
                ps = mm_ps.tile([128, 512], f32, name="v_ps", tag="mm")
                for k in range(4):
                    nc.tensor.matmul(
                        ps, hn[k][:, 128 * m:128 * (m + 1)], wv[k],
                        start=(k == 0), stop=False)
                nc.tensor.matmul(ps, ones_row, bv, start=False, stop=True)
                nc.vector.tensor_copy(out=vT[m], in_=ps)

            def emit_attention(p):
                qp, kp, ekp = qk[2 * p], qk[2 * p + 1], ek[p]
                av = [av_ps.tile([128, 512], f32, name=f"av{n}", tag="av")
                      for n in range(2)]
                dps = mm_ps.tile([128, 512], f32, name="dps", tag="mm")
                nchunks = len(S_CHUNKS)
                for ci, (s0, sw) in enumerate(S_CHUNKS):
                    first, last = ci == 0, ci == nchunks - 1
                    pT = []
                    for hh in range(2):
                        pb = 64 * hh
                        st = st_ps.tile([128, 1024], f32, name="st", tag="st")
                        if first:
                            lhsT = ekp[pb:pb + 64, :]
                        else:
                            lhsT = kp[pb:pb + 64, s0 - ENC_L:s0 - ENC_L + sw]
                        for n in range(2):
                            nc.tensor.matmul(
                                st[0:sw, 512 * n:512 * (n + 1)],
                                lhsT, qp[pb:pb + 64, 512 * n:512 * (n + 1)],
                                start=True, stop=True)
                        pt = pts.tile([128, 1024], bf, name="pt", tag="pt")
                        nc.scalar.activation(out=pt[0:sw, :], in_=st[0:sw, :], func=AF.Exp)
                        pT.append(pt)
                    for n in range(2):
                        for hh in range(2):
                            vslice = (evT if first else vT[ci - 1])[
                                0:sw, 64 * (2 * p + hh):64 * (2 * p + hh) + 64]
                            nc.tensor.matmul(
                                av[n][64 * hh:64 * hh + 64, :],
                                vslice, pT[hh][0:sw, 512 * n:512 * (n + 1)],
                                start=first, stop=last,
                                skip_group_check=True)
                    for hh in range(2):
                        for n in range(2):
                            j = 2 * hh + n
                            nc.tensor.matmul(
                                dps[32 * j:32 * j + 1, :],
                                ones_col[0:sw, :],
                                pT[hh][0:sw, 512 * n:512 * (n + 1)],
                                start=first, stop=last,
                                skip_group_check=True, tile_position=(0, 32 * j))
                # free PSUM early: copy unnormalized accumulators to SBUF
                avr = pts.tile([128, 1024], f32, name="avr", tag="avr", bufs=2)
                for n in range(2):
                    nc.vector.tensor_copy(out=avr[:, 512 * n:512 * (n + 1)], in_=av[n])
                dsb = small.tile([128, 512], f32, name="dsb", tag="dsb", bufs=2)
                nc.vector.tensor_copy(out=dsb, in_=dps)
                nc.vector.reciprocal(out=dsb, in_=dsb)
                ddr = ddr_pool.tile([4, 512], f32, name="ddr", tag="ddr")
                nc.sync.dma_start(out=ddr[0:4, :], in_=dsb[::32, :])
                dbc = [pts.tile([128, 512], f32, name=f"dbc{n}", tag=f"dbc{n}",
                                bufs=1) for n in range(2)]
                for hh in range(2):
                    for n in range(2):
                        j = 2 * hh + n
                        src = bass.AP(tensor=ddr.tensor, offset=512 * j,
                                      ap=[[0, 64], [1, 512]])
                        nc.sync.dma_start(
                            out=dbc[n][64 * hh:64 * hh + 64, :], in_=src)
                for n in range(2):
                    nc.vector.tensor_tensor(
                        out=a_sb[p][:, 512 * n:512 * (n + 1)],
                        in0=avr[:, 512 * n:512 * (n + 1)],
                        in1=dbc[n], op=OP.mult)

            with nc.named_scope("qkv"):
                emit_qk(0)
                emit_qk(1)
                for m in range(8):
                    emit_vT(m)
            with nc.named_scope("attn"):
                for p in range(4):
                    emit_attention(p)
                    if p < 3:
                        with nc.named_scope("qkv"):
                            emit_qk(2 * p + 2)
                            emit_qk(2 * p + 3)

            # ---------------- proj + residual ----------------
            with nc.named_scope("proj"):
                for m in range(4):
                    for n in range(2):
                        if (2 * m + n) % 2 == 0:
                            ps = mm_ps.tile([128, 512], f32, name="pj_ps", tag="mm")
                        else:
                            ps = av_ps.tile([128, 512], f32, name="pj_ps2", tag="av")
                        for k in range(4):
                            nc.tensor.matmul(
                                ps, wp[k][:, 128 * m:128 * (m + 1)],
                                a_sb[k][:, 512 * n:512 * (n + 1)],
                                start=(k == 0), stop=(k == 3))
                        ot = data.tile([128, 512], f32, name="ot", tag="ot", bufs=2)
                        nc.vector.scalar_tensor_tensor(
                            out=ot, in0=ps, scalar=bp[:, m:m + 1],
                            in1=xt[m][:, 512 * n:512 * (n + 1)],
                            op0=OP.add, op1=OP.add)
                        eng = nc.sync if (2 * m + n) % 2 == 0 else nc.gpsimd
                        eng.dma_start(
                            out=out_d[128 * m:128 * (m + 1), 512 * n:512 * (n + 1)], in_=ot)
    nc.compile()
    return nc


def _host_prep(x, encoder_out, gn_w, gn_b, qkv_w, qkv_b, ekv_w, ekv_b, proj_w, proj_b):
    """Build per-core in_maps (weights replicated, batch sharded)."""
    x = np.asarray(x, np.float32).reshape(B, C, L)
    enc = np.asarray(encoder_out, np.float32)
    qkv_w = np.asarray(qkv_w, np.float32); qkv_b = np.asarray(qkv_b, np.float32)
    ekv_w = np.asarray(ekv_w, np.float32); ekv_b = np.asarray(ekv_b, np.float32)
    proj_w = np.asarray(proj_w, np.float32); proj_b = np.asarray(proj_b, np.float32)
    gn_w = np.asarray(gn_w, np.float32); gn_b = np.asarray(gn_b, np.float32)

    qk_order, v_order, ek_order, ev_order = [], [], [], []
    for p in range(4):
        for h in (2 * p, 2 * p + 1):
            qk_order += [192 * h + i for i in range(64)]
        for h in (2 * p, 2 * p + 1):
            qk_order += [192 * h + 64 + i for i in range(64)]
        for h in (2 * p, 2 * p + 1):
            ek_order += [128 * h + i for i in range(64)]
    for h in range(8):
        v_order += [192 * h + 128 + i for i in range(64)]
        ev_order += [128 * h + 64 + i for i in range(64)]

    wqk = (qkv_w[qk_order, :].T * SCALE).astype(BF16)
    bqk = (qkv_b[qk_order] * SCALE).astype(np.float32).reshape(8, 128).T.copy()
    wv = qkv_w[v_order, :].T.astype(BF16)
    bv = qkv_b[v_order].astype(BF16).reshape(1, 512)
    wek = (ekv_w[ek_order, :].T * SCALE).astype(BF16)
    bek = (ekv_b[ek_order] * SCALE).astype(np.float32).reshape(4, 128).T.copy()
    wev = ekv_w[ev_order, :].T.astype(BF16)
    bev = ekv_b[ev_order].astype(BF16).reshape(1, 512)
    wp = proj_w.T.astype(BF16)
    bp = proj_b.astype(np.float32).reshape(4, 128).T.copy()
    gnw4 = gn_w.reshape(4, 128).T.copy()
    gnb4 = gn_b.reshape(4, 128).T.copy()
    emat = np.zeros((128, 8), BF16)
    for pp in range(128):
        emat[pp, pp // 16] = 1
    etmat = np.ascontiguousarray(emat.T)

    shared = dict(
        wqk=np.ascontiguousarray(wqk), wv=np.ascontiguousarray(wv),
        wek=np.ascontiguousarray(wek), wev=np.ascontiguousarray(wev),
        wp=np.ascontiguousarray(wp),
        bqk=np.ascontiguousarray(bqk), bek=np.ascontiguousarray(bek),
        bv=bv, bev=bev, bp=np.ascontiguousarray(bp),
        gnw=np.ascontiguousarray(gnw4), gnb=np.ascontiguousarray(gnb4),
        emat=emat, etmat=etmat,
    )
    in_maps = []
    for b in range(B):
        m = dict(shared)
        m["x"] = np.ascontiguousarray(x[b])
        m["enc"] = np.ascontiguousarray(enc[b].astype(BF16))
        in_maps.append(m)
    return in_maps


_NC_CACHE = {}


def _get_nc():
    if "nc" not in _NC_CACHE:
        _NC_CACHE["nc"] = _build_bass()
    return _NC_CACHE["nc"]


def kernel(**inputs):
    from concourse.bass_utils import run_bass_kernel_spmd
    in_maps = _host_prep(**inputs)
    nc = _get_nc()
    res = run_bass_kernel_spmd(nc, in_maps, core_ids=list(range(N_CORES)))
    out = np.stack([res.results[b]["out"] for b in range(B)])
    return out.reshape(B, C, H, W).astype(np.float32)



# revision 4
# speedup vs baseline: 1.1892x; 1.1892x over previous
"""AttentionBlock Trainium2 kernel (nn_AttentionBlock dense_transformer).

Sharding: data-parallel over batch B=8 across 8 NeuronCores (1 image/core).
Per-core pipeline:
  - GroupNorm(32 groups) over x [512, 1024]
  - qkv / encoder_kv projections (bf16 matmuls, fp32 PSUM accumulate)
      q,k in [c, t] layout (orientation A), v/ev transposed [s, c] (orientation B)
  - attention: S^T = k^T q in [s, t] layout; softmax axis = partitions.
      Max-subtraction is skipped (logits are O(6) by construction: normalized
      activations x unit-variance weights, scale folded on host).
      exp on ScalarE; A = sum_s P v via matmuls whose lhsT (V) carries an
      extra ones-column per head, so PSUM row 64 accumulates the softmax
      denominator D for free (matmul cost depends only on the moving dim).
      P/D applied during PSUM->SBUF copy via a DRAM-broadcast of 1/D.
  - proj + residual add
All matmul inputs bf16 (fp32 accumulation); end-to-end error vs fp32 reference
measured ~5e-4 relative.
"""

import numpy as np
import ml_dtypes

B, C, H, W = 8, 512, 32, 32
L = H * W                      # 1024
NH = 8
CH = C // NH                   # 64 per head
G = 32                         # groupnorm groups
GS = C // G                    # 16 channels per group
ENC_C, ENC_L = 768, 77
EPS = 1e-5
S_TOT = ENC_L + L              # 1101
SCALE = 1.0 / np.sqrt(np.sqrt(CH))
N_CORES = 8

# s-chunks of the key/value axis: enc block (77) then 8 x 128 self blocks
S_CHUNKS = [(0, ENC_L)] + [(ENC_L + 128 * i, 128) for i in range(8)]

BF16 = ml_dtypes.bfloat16


def _build_bass(debug=False):
    import concourse.bass as bass
    import concourse.mybir as mybir
    import concourse.tile as tile
    from concourse import bacc

    f32 = mybir.dt.float32
    bf = mybir.dt.bfloat16
    AF = mybir.ActivationFunctionType
    OP = mybir.AluOpType

    nc = bacc.Bacc()

    # ---- DRAM I/O ----
    x_d = nc.dram_tensor("x", [C, L], f32, kind="ExternalInput")
    enc_d = nc.dram_tensor("enc", [ENC_C, ENC_L], bf, kind="ExternalInput")
    wqk_d = nc.dram_tensor("wqk", [C, 1024], bf, kind="ExternalInput")
    wv_d = nc.dram_tensor("wv", [C, 512], bf, kind="ExternalInput")
    wek_d = nc.dram_tensor("wek", [ENC_C, 512], bf, kind="ExternalInput")
    wev_d = nc.dram_tensor("wev", [ENC_C, 512], bf, kind="ExternalInput")
    wp_d = nc.dram_tensor("wp", [C, C], bf, kind="ExternalInput")
    bqk_d = nc.dram_tensor("bqk", [128, 8], f32, kind="ExternalInput")
    bek_d = nc.dram_tensor("bek", [128, 4], f32, kind="ExternalInput")
    bv_d = nc.dram_tensor("bv", [1, 512], bf, kind="ExternalInput")
    bev_d = nc.dram_tensor("bev", [1, 512], bf, kind="ExternalInput")
    bp_d = nc.dram_tensor("bp", [128, 4], f32, kind="ExternalInput")
    gnw_d = nc.dram_tensor("gnw", [128, 4], f32, kind="ExternalInput")
    gnb_d = nc.dram_tensor("gnb", [128, 4], f32, kind="ExternalInput")
    emat_d = nc.dram_tensor("emat", [128, 8], bf, kind="ExternalInput")
    etmat_d = nc.dram_tensor("etmat", [8, 128], bf, kind="ExternalInput")
    out_d = nc.dram_tensor("out", [C, L], f32, kind="ExternalOutput")

    with tile.TileContext(nc) as tc:
        with tc.tile_pool(name="wpool", bufs=1) as wpool, \
             tc.tile_pool(name="data", bufs=1) as data, \
             tc.tile_pool(name="small", bufs=1) as small, \
             tc.tile_pool(name="pts", bufs=6) as pts, \
             tc.tile_pool(name="ddr", bufs=2, space="DRAM") as ddr_pool, \
             tc.tile_pool(name="big_ps", bufs=2, space="PSUM") as big_ps, \
             tc.tile_pool(name="av_ps", bufs=1, space="PSUM") as av_ps:

            # ---------------- loads, in consumption order ----------------
            xt = [data.tile([128, 1024], f32, name=f"xt{k}") for k in range(4)]
            for k in range(4):
                eng = nc.sync if k % 2 == 0 else nc.gpsimd
                eng.dma_start(out=xt[k], in_=x_d[128 * k:128 * (k + 1), :])
            enct = [data.tile([128, ENC_L], bf, name=f"enct{k}") for k in range(6)]
            for k in range(6):
                nc.sync.dma_start(out=enct[k], in_=enc_d[128 * k:128 * (k + 1), :])
            wek = [wpool.tile([128, 512], bf, name=f"wek{k}") for k in range(6)]
            wev = [wpool.tile([128, 512], bf, name=f"wev{k}") for k in range(6)]
            for k in range(6):
                nc.sync.dma_start(out=wek[k], in_=wek_d[128 * k:128 * (k + 1), :])
                nc.gpsimd.dma_start(out=wev[k], in_=wev_d[128 * k:128 * (k + 1), :])
            wqk = [wpool.tile([128, 1024], bf, name=f"wqk{k}") for k in range(4)]
            for k in range(4):
                nc.gpsimd.dma_start(out=wqk[k], in_=wqk_d[128 * k:128 * (k + 1), :])
            wv = [wpool.tile([128, 512], bf, name=f"wv{k}") for k in range(4)]
            for k in range(4):
                nc.gpsimd.dma_start(out=wv[k], in_=wv_d[128 * k:128 * (k + 1), :])
            wp = [wpool.tile([128, 512], bf, name=f"wp{k}") for k in range(4)]
            for k in range(4):
                nc.gpsimd.dma_start(out=wp[k], in_=wp_d[128 * k:128 * (k + 1), :])
            bqk = wpool.tile([128, 8], f32)
            nc.sync.dma_start(out=bqk, in_=bqk_d[:, :])
            bek = wpool.tile([128, 4], f32)
            nc.sync.dma_start(out=bek, in_=bek_d[:, :])
            bv = wpool.tile([1, 512], bf)
            nc.sync.dma_start(out=bv, in_=bv_d[:, :])
            bev = wpool.tile([1, 512], bf)
            nc.sync.dma_start(out=bev, in_=bev_d[:, :])
            bp = wpool.tile([128, 4], f32)
            nc.sync.dma_start(out=bp, in_=bp_d[:, :])
            gnw = wpool.tile([128, 4], f32)
            nc.sync.dma_start(out=gnw, in_=gnw_d[:, :])
            gnb = wpool.tile([128, 4], f32)
            nc.sync.dma_start(out=gnb, in_=gnb_d[:, :])
            emat = wpool.tile([128, 8], bf)
            nc.sync.dma_start(out=emat, in_=emat_d[:, :])
            etmat = wpool.tile([8, 128], bf)
            nc.sync.dma_start(out=etmat, in_=etmat_d[:, :])

            ones_row = wpool.tile([1, 128], bf)   # lhsT for K=1 bias matmuls
            nc.vector.memset(ones_row, 1.0)

            # V tiles: [s, 8 heads x (64 ch + ones col)].  The ones column per
            # head makes the AV matmul also accumulate the softmax denominator
            # into PSUM row 64 (lhsT columns = out partitions; cost unchanged).
            vT = [data.tile([128, 520], bf, name=f"vT{m}") for m in range(8)]
            evT = data.tile([ENC_L, 520], bf)
            for m in range(8):
                nc.gpsimd.memset(vT[m][:, 64::65], 1.0)
            nc.gpsimd.memset(evT[:, 64::65], 1.0)

            def blocks_ap(t, npart):
                full = t[:, :]
                return bass.AP(tensor=full.tensor, offset=full.offset,
                               ap=[[520, npart], [65, 8], [1, 64]])

            # ---------------- encoder kv (small, first) ----------------
            ek = [data.tile([128, ENC_L], bf, name=f"ek{p}") for p in range(4)]
            with nc.named_scope("ekv"):
                for half in range(2):
                    ps = big_ps.tile([128, 1024], f32, name="ek_ps", tag="st")
                    for i in range(2):
                        p = 2 * half + i
                        for k in range(6):
                            nc.tensor.matmul(
                                ps[:, 512 * i:512 * i + ENC_L],
                                wek[k][:, 128 * p:128 * (p + 1)], enct[k],
                                start=(k == 0), stop=(k == 5))
                    for i in range(2):
                        p = 2 * half + i
                        nc.vector.tensor_scalar_add(
                            out=ek[p], in0=ps[:, 512 * i:512 * i + ENC_L],
                            scalar1=bek[:, p:p + 1])
                ps = big_ps.tile([128, 1024], f32, name="ev_ps", tag="st")
                for k in range(6):
                    nc.tensor.matmul(ps[0:ENC_L, 0:512], enct[k], wev[k],
                                     start=(k == 0), stop=False)
                nc.tensor.matmul(ps[0:ENC_L, 0:512], ones_row[:, 0:ENC_L], bev,
                                 start=False, stop=True)
                nc.vector.tensor_copy(out=blocks_ap(evT, ENC_L),
                                      in_=ps[0:ENC_L, 0:512])

            # ---------------- GroupNorm ----------------
            with nc.named_scope("gn"):
                stats = small.tile([128, 8], f32)
                for k in range(4):
                    nc.vector.reduce_sum(stats[:, k:k + 1], xt[k], axis=mybir.AxisListType.X)
                for k in range(4):
                    xsq = small.tile([128, 1024], f32, name="xsq", tag="xsq", bufs=2)
                    nc.scalar.activation(out=xsq, in_=xt[k], func=AF.Square,
                                         accum_out=stats[:, 4 + k:5 + k])
                stats_bf = small.tile([128, 8], bf)
                nc.vector.tensor_copy(out=stats_bf, in_=stats)
                gps = big_ps.tile([128, 1024], f32, name="gn_ps", tag="st")
                g8_ps = gps[0:8, 0:8]
                nc.tensor.matmul(g8_ps, emat, stats_bf, start=True, stop=True)
                musg = small.tile([8, 8], f32)   # cols 0:4 mean, 4:8 later rstd
                inv_n = 1.0 / (GS * L)
                nc.vector.tensor_scalar_mul(out=musg, in0=g8_ps, scalar1=inv_n)
                var8 = small.tile([8, 4], f32)
                nc.vector.tensor_mul(out=var8, in0=musg[:, 0:4], in1=musg[:, 0:4])
                nc.vector.tensor_sub(out=var8, in0=musg[:, 4:8], in1=var8)
                epst = small.tile([8, 1], f32)
                nc.vector.memset(epst, EPS)
                lnv = small.tile([8, 4], f32)
                nc.scalar.activation(out=lnv, in_=var8, func=AF.Ln, bias=epst, scale=1.0)
                nc.scalar.activation(out=musg[:, 4:8], in_=lnv, func=AF.Exp, scale=-0.5)
                musg_bf = small.tile([8, 8], bf)
                nc.vector.tensor_copy(out=musg_bf, in_=musg)
                exp_ps = gps[:, 512:520]
                nc.tensor.matmul(exp_ps, etmat, musg_bf, start=True, stop=True)
                aff_a = small.tile([128, 4], f32)
                nc.vector.tensor_mul(out=aff_a, in0=gnw, in1=exp_ps[:, 4:8])
                aff_b = small.tile([128, 4], f32)
                nc.vector.tensor_mul(out=aff_b, in0=exp_ps[:, 0:4], in1=aff_a)
                nc.vector.tensor_sub(out=aff_b, in0=gnb, in1=aff_b)
                hn = [data.tile([128, 1024], bf, name=f"hn{k}") for k in range(4)]
                for k in range(4):
                    nc.gpsimd.tensor_scalar(
                        out=hn[k], in0=xt[k], scalar1=aff_a[:, k:k + 1],
                        scalar2=aff_b[:, k:k + 1], op0=OP.mult, op1=OP.add)

            # ---------------- projections + attention, interleaved ----------------
            qk = [data.tile([128, 1024], bf, name=f"qk{m}") for m in range(8)]
            a_sb = [data.tile([128, 1024], bf, name=f"a_sb{p}") for p in range(4)]

            def emit_qk(m):
                ps = big_ps.tile([128, 1024], f32, name="qkv_ps", tag="st")
                for n in range(2):
                    for k in range(4):
                        nc.tensor.matmul(
                            ps[:, 512 * n:512 * (n + 1)],
                            wqk[k][:, 128 * m:128 * (m + 1)],
                            hn[k][:, 512 * n:512 * (n + 1)],
                            start=(k == 0), stop=(k == 3))
                nc.vector.tensor_scalar_add(out=qk[m], in0=ps, scalar1=bqk[:, m:m + 1])

            def emit_v2(m0):
                ps = big_ps.tile([128, 1024], f32, name="v_ps", tag="st")
                for i in range(2):
                    m = m0 + i
                    for k in range(4):
                        nc.tensor.matmul(
                            ps[:, 512 * i:512 * (i + 1)],
                            hn[k][:, 128 * m:128 * (m + 1)], wv[k],
                            start=(k == 0), stop=False)
                    nc.tensor.matmul(ps[:, 512 * i:512 * (i + 1)], ones_row, bv,
                                     start=False, stop=True)
                for i in range(2):
                    m = m0 + i
                    nc.vector.tensor_copy(out=blocks_ap(vT[m], 128),
                                          in_=ps[:, 512 * i:512 * (i + 1)])

            def emit_attention(p, interleave_v=False):
                qp, kp, ekp = qk[2 * p], qk[2 * p + 1], ek[p]
                avt = [[av_ps.tile([128, 512], f32, name=f"av{hh}{n}",
                                   tag=f"av{hh}{n}") for n in range(2)]
                       for hh in range(2)]
                nchunks = len(S_CHUNKS)
                for ci, (s0, sw) in enumerate(S_CHUNKS):
                    first, last = ci == 0, ci == nchunks - 1
                    pT = []
                    for hh in range(2):
                        pb = 64 * hh
                        st = big_ps.tile([128, 1024], f32, name="st", tag="st")
                        if first:
                            lhsT = ekp[pb:pb + 64, :]
                        else:
                            lhsT = kp[pb:pb + 64, s0 - ENC_L:s0 - ENC_L + sw]
                        for n in range(2):
                            nc.tensor.matmul(
                                st[0:sw, 512 * n:512 * (n + 1)],
                                lhsT, qp[pb:pb + 64, 512 * n:512 * (n + 1)],
                                start=True, stop=True)
                        pt = pts.tile([128, 1024], bf, name="pt", tag="pt")
                        nc.scalar.activation(out=pt[0:sw, :], in_=st[0:sw, :], func=AF.Exp)
                        pT.append(pt)
                    vsrc = evT if first else vT[ci - 1]
                    for hh in range(2):
                        h = 2 * p + hh
                        lhsTv = vsrc[0:sw, 65 * h:65 * h + 65]
                        for n in range(2):
                            nc.tensor.matmul(
                                avt[hh][n][0:65, :],
                                lhsTv, pT[hh][0:sw, 512 * n:512 * (n + 1)],
                                start=first, stop=last,
                                skip_group_check=True)
                    if interleave_v and ci in (1, 3, 5):
                        emit_v2(ci + 1)
                # normalization: A[c,t] / D[t], D in PSUM row 64 of each av tile
                dsb = small.tile([128, 512], f32, name="dsb", tag="dsb", bufs=2)
                for hh in range(2):
                    for n in range(2):
                        j = 2 * hh + n
                        nc.vector.reciprocal(out=dsb[32 * j:32 * j + 1, :],
                                             in_=avt[hh][n][64:65, :])
                ddr = ddr_pool.tile([4, 512], f32, name="ddr", tag="ddr")
                nc.sync.dma_start(out=ddr[0:4, :], in_=dsb[::32, :])
                dbc = [pts.tile([128, 512], f32, name=f"dbc{n}", tag=f"dbc{n}",
                                bufs=2) for n in range(2)]
                for hh in range(2):
                    for n in range(2):
                        j = 2 * hh + n
                        src = bass.AP(tensor=ddr.tensor, offset=512 * j,
                                      ap=[[0, 64], [1, 512]])
                        nc.sync.dma_start(
                            out=dbc[n][64 * hh:64 * hh + 64, :], in_=src)
                for hh in range(2):
                    for n in range(2):
                        nc.vector.tensor_tensor(
                            out=a_sb[p][64 * hh:64 * hh + 64, 512 * n:512 * (n + 1)],
                            in0=avt[hh][n][0:64, :],
                            in1=dbc[n][64 * hh:64 * hh + 64, :], op=OP.mult)

            with nc.named_scope("qkv"):
                emit_qk(0)
                emit_qk(1)
                emit_v2(0)
            with nc.named_scope("attn"):
                for p in range(4):
                    emit_attention(p, interleave_v=(p == 0))
                    if p < 3:
                        with nc.named_scope("qkv"):
                            emit_qk(2 * p + 2)
                            emit_qk(2 * p + 3)

            # ---------------- proj + residual ----------------
            with nc.named_scope("proj"):
                for m in range(4):
                    ps = big_ps.tile([128, 1024], f32, name="pj_ps", tag="st")
                    for n in range(2):
                        for k in range(4):
                            nc.tensor.matmul(
                                ps[:, 512 * n:512 * (n + 1)],
                                wp[k][:, 128 * m:128 * (m + 1)],
                                a_sb[k][:, 512 * n:512 * (n + 1)],
                                start=(k == 0), stop=(k == 3))
                    ot = data.tile([128, 1024], f32, name="ot", tag="ot", bufs=2)
                    nc.vector.scalar_tensor_tensor(
                        out=ot, in0=ps, scalar=bp[:, m:m + 1],
                        in1=xt[m], op0=OP.add, op1=OP.add)
                    deng = nc.sync if m % 2 == 0 else nc.gpsimd
                    deng.dma_start(out=out_d[128 * m:128 * (m + 1), :], in_=ot)
    nc.compile()
    return nc


def _host_prep(x, encoder_out, gn_w, gn_b, qkv_w, qkv_b, ekv_w, ekv_b, proj_w, proj_b):
    """Build per-core in_maps (weights replicated, batch sharded)."""
    x = np.asarray(x, np.float32).reshape(B, C, L)
    enc = np.asarray(encoder_out, np.float32)
    qkv_w = np.asarray(qkv_w, np.float32); qkv_b = np.asarray(qkv_b, np.float32)
    ekv_w = np.asarray(ekv_w, np.float32); ekv_b = np.asarray(ekv_b, np.float32)
    proj_w = np.asarray(proj_w, np.float32); proj_b = np.asarray(proj_b, np.float32)
    gn_w = np.asarray(gn_w, np.float32); gn_b = np.asarray(gn_b, np.float32)

    qk_order, v_order, ek_order, ev_order = [], [], [], []
    for p in range(4):
        for h in (2 * p, 2 * p + 1):
            qk_order += [192 * h + i for i in range(64)]
        for h in (2 * p, 2 * p + 1):
            qk_order += [192 * h + 64 + i for i in range(64)]
        for h in (2 * p, 2 * p + 1):
            ek_order += [128 * h + i for i in range(64)]
    for h in range(8):
        v_order += [192 * h + 128 + i for i in range(64)]
        ev_order += [128 * h + 64 + i for i in range(64)]

    wqk = (qkv_w[qk_order, :].T * SCALE).astype(BF16)
    bqk = (qkv_b[qk_order] * SCALE).astype(np.float32).reshape(8, 128).T.copy()
    wv = qkv_w[v_order, :].T.astype(BF16)
    bv = qkv_b[v_order].astype(BF16).reshape(1, 512)
    wek = (ekv_w[ek_order, :].T * SCALE).astype(BF16)
    bek = (ekv_b[ek_order] * SCALE).astype(np.float32).reshape(4, 128).T.copy()
    wev = ekv_w[ev_order, :].T.astype(BF16)
    bev = ekv_b[ev_order].astype(BF16).reshape(1, 512)
    wp = proj_w.T.astype(BF16)
    bp = proj_b.astype(np.float32).reshape(4, 128).T.copy()
    gnw4 = gn_w.reshape(4, 128).T.copy()
    gnb4 = gn_b.reshape(4, 128).T.copy()
    emat = np.zeros((128, 8), BF16)
    for pp in range(128):
        emat[pp, pp // 16] = 1
    etmat = np.ascontiguousarray(emat.T)

    shared = dict(
        wqk=np.ascontiguousarray(wqk), wv=np.ascontiguousarray(wv),
        wek=np.ascontiguousarray(wek), wev=np.ascontiguousarray(wev),
        wp=np.ascontiguousarray(wp),
        bqk=np.ascontiguousarray(bqk), bek=np.ascontiguousarray(bek),
        bv=bv, bev=bev, bp=np.ascontiguousarray(bp),
        gnw=np.ascontiguousarray(gnw4), gnb=np.ascontiguousarray(gnb4),
        emat=emat, etmat=etmat,
    )
    in_maps = []
    for b in range(B):
        m = dict(shared)
        m["x"] = np.ascontiguousarray(x[b])
        m["enc"] = np.ascontiguousarray(enc[b].astype(BF16))
        in_maps.append(m)
    return in_maps


_NC_CACHE = {}


def _get_nc():
    if "nc" not in _NC_CACHE:
        _NC_CACHE["nc"] = _build_bass()
    return _NC_CACHE["nc"]


def kernel(**inputs):
    from concourse.bass_utils import run_bass_kernel_spmd
    in_maps = _host_prep(**inputs)
    nc = _get_nc()
    res = run_bass_kernel_spmd(nc, in_maps, core_ids=list(range(N_CORES)))
    out = np.stack([res.results[b]["out"] for b in range(B)])
    return out.reshape(B, C, H, W).astype(np.float32)


# revision 7
# speedup vs baseline: 1.2120x; 1.0191x over previous
"""AttentionBlock Trainium2 kernel (nn_AttentionBlock dense_transformer).

Sharding: data-parallel over batch B=8 across 8 NeuronCores (1 image/core).
Per-core pipeline:
  - GroupNorm(32 groups) over x [512, 1024]
  - qkv / encoder_kv projections (bf16 matmuls, fp32 PSUM accumulate)
      q,k in [c, t] layout (orientation A), v/ev transposed [s, c] (orientation B)
  - attention: S^T = k^T q in [s, t] layout; softmax axis = partitions.
      Max-subtraction is skipped (logits are O(6) by construction: normalized
      activations x unit-variance weights, scale folded on host).
      exp on ScalarE; A = sum_s P v via matmuls whose lhsT (V) carries an
      extra ones-column per head, so PSUM row 64 accumulates the softmax
      denominator D for free (matmul cost depends only on the moving dim).
      P/D applied during PSUM->SBUF copy via a DRAM-broadcast of 1/D.
  - proj + residual add
All matmul inputs bf16 (fp32 accumulation); end-to-end error vs fp32 reference
measured ~5e-4 relative.
"""

import numpy as np
import ml_dtypes

B, C, H, W = 8, 512, 32, 32
L = H * W                      # 1024
NH = 8
CH = C // NH                   # 64 per head
G = 32                         # groupnorm groups
GS = C // G                    # 16 channels per group
ENC_C, ENC_L = 768, 77
EPS = 1e-5
S_TOT = ENC_L + L              # 1101
SCALE = 1.0 / np.sqrt(np.sqrt(CH))
N_CORES = 8

# s-chunks of the key/value axis: enc block (77) then 8 x 128 self blocks
S_CHUNKS = [(0, ENC_L)] + [(ENC_L + 128 * i, 128) for i in range(8)]

BF16 = ml_dtypes.bfloat16


def _build_bass(debug=False):
    import concourse.bass as bass
    import concourse.mybir as mybir
    import concourse.tile as tile
    from concourse import bacc

    f32 = mybir.dt.float32
    bf = mybir.dt.bfloat16
    AF = mybir.ActivationFunctionType
    OP = mybir.AluOpType

    nc = bacc.Bacc()

    # ---- DRAM I/O ----
    x_d = nc.dram_tensor("x", [C, L], f32, kind="ExternalInput")
    enc_d = nc.dram_tensor("enc", [ENC_C, ENC_L], bf, kind="ExternalInput")
    wqk_d = nc.dram_tensor("wqk", [C, 1024], bf, kind="ExternalInput")
    wv_d = nc.dram_tensor("wv", [C, 512], bf, kind="ExternalInput")
    wek_d = nc.dram_tensor("wek", [ENC_C, 512], bf, kind="ExternalInput")
    wev_d = nc.dram_tensor("wev", [ENC_C, 512], bf, kind="ExternalInput")
    wp_d = nc.dram_tensor("wp", [C, C], bf, kind="ExternalInput")
    bqk_d = nc.dram_tensor("bqk", [128, 8], f32, kind="ExternalInput")
    bek_d = nc.dram_tensor("bek", [128, 4], f32, kind="ExternalInput")
    bv_d = nc.dram_tensor("bv", [1, 512], bf, kind="ExternalInput")
    bev_d = nc.dram_tensor("bev", [1, 512], bf, kind="ExternalInput")
    bp_d = nc.dram_tensor("bp", [128, 4], f32, kind="ExternalInput")
    gnw_d = nc.dram_tensor("gnw", [128, 4], f32, kind="ExternalInput")
    gnb_d = nc.dram_tensor("gnb", [128, 4], f32, kind="ExternalInput")
    emat_d = nc.dram_tensor("emat", [128, 8], bf, kind="ExternalInput")
    etmat_d = nc.dram_tensor("etmat", [8, 128], bf, kind="ExternalInput")
    out_d = nc.dram_tensor("out", [C, L], f32, kind="ExternalOutput")

    with tile.TileContext(nc) as tc:
        with tc.tile_pool(name="wpool", bufs=1) as wpool, \
             tc.tile_pool(name="data", bufs=1) as data, \
             tc.tile_pool(name="small", bufs=1) as small, \
             tc.tile_pool(name="pts", bufs=6) as pts, \
             tc.tile_pool(name="ddr", bufs=2, space="DRAM") as ddr_pool, \
             tc.tile_pool(name="big_ps", bufs=2, space="PSUM") as big_ps, \
             tc.tile_pool(name="av_ps", bufs=1, space="PSUM") as av_ps:

            # ---------------- loads, in consumption order ----------------
            xt = [data.tile([128, 1024], f32, name=f"xt{k}") for k in range(4)]
            for k in range(4):
                eng = nc.sync if k % 2 == 0 else nc.gpsimd
                eng.dma_start(out=xt[k], in_=x_d[128 * k:128 * (k + 1), :])
            enct = [data.tile([128, ENC_L], bf, name=f"enct{k}") for k in range(6)]
            for k in range(6):
                nc.sync.dma_start(out=enct[k], in_=enc_d[128 * k:128 * (k + 1), :])
            wek = [wpool.tile([128, 512], bf, name=f"wek{k}") for k in range(6)]
            wev = [wpool.tile([128, 512], bf, name=f"wev{k}") for k in range(6)]
            for k in range(6):
                nc.sync.dma_start(out=wek[k], in_=wek_d[128 * k:128 * (k + 1), :])
                nc.gpsimd.dma_start(out=wev[k], in_=wev_d[128 * k:128 * (k + 1), :])
            wqk = [wpool.tile([128, 1024], bf, name=f"wqk{k}") for k in range(4)]
            for k in range(4):
                nc.gpsimd.dma_start(out=wqk[k], in_=wqk_d[128 * k:128 * (k + 1), :])
            wv = [wpool.tile([128, 512], bf, name=f"wv{k}") for k in range(4)]
            for k in range(4):
                nc.gpsimd.dma_start(out=wv[k], in_=wv_d[128 * k:128 * (k + 1), :])
            wp = [wpool.tile([128, 512], bf, name=f"wp{k}") for k in range(4)]
            for k in range(4):
                nc.gpsimd.dma_start(out=wp[k], in_=wp_d[128 * k:128 * (k + 1), :])
            bqk = wpool.tile([128, 8], f32)
            nc.sync.dma_start(out=bqk, in_=bqk_d[:, :])
            bek = wpool.tile([128, 4], f32)
            nc.sync.dma_start(out=bek, in_=bek_d[:, :])
            bp = wpool.tile([128, 4], f32)
            nc.sync.dma_start(out=bp, in_=bp_d[:, :])
            gnw = wpool.tile([128, 4], f32)
            nc.sync.dma_start(out=gnw, in_=gnw_d[:, :])
            gnb = wpool.tile([128, 4], f32)
            nc.sync.dma_start(out=gnb, in_=gnb_d[:, :])
            emat = wpool.tile([128, 8], bf)
            nc.sync.dma_start(out=emat, in_=emat_d[:, :])
            etmat = wpool.tile([8, 128], bf)
            nc.sync.dma_start(out=etmat, in_=etmat_d[:, :])

            # Warm the activation table once (Exp then Ln narrows the sim to
            # the natural_log_exp set, which also serves Square/Copy): all
            # later activations avoid the 1.3us ACT_TABLE_LOAD.
            wtiny = small.tile([1, 8], f32)
            nc.vector.memset(wtiny, 1.0)
            wout = small.tile([1, 8], f32)
            nc.scalar.activation(out=wout, in_=wtiny, func=AF.Exp)
            nc.scalar.activation(out=wout, in_=wtiny, func=AF.Ln)

            # per-channel v / ev biases broadcast to all 128 partitions (DMA
            # partition-stride-0), added during the PSUM->SBUF v copies.
            vbias_bc = wpool.tile([128, 512], bf)
            evbias_bc = wpool.tile([128, 512], bf)
            for t, d in ((vbias_bc, bv_d), (evbias_bc, bev_d)):
                fd = d[:, :]
                nc.sync.dma_start(out=t, in_=bass.AP(
                    tensor=fd.tensor, offset=fd.offset, ap=[[0, 128], [1, 512]]))

            # V tiles: [s, 8 heads x (64 ch + ones col)].  The ones column per
            # head makes the AV matmul also accumulate the softmax denominator
            # into PSUM row 64 (lhsT columns = out partitions; cost unchanged).
            vT = [data.tile([128, 520], bf, name=f"vT{m}") for m in range(8)]
            evT = data.tile([ENC_L, 520], bf)
            for m in range(8):
                nc.gpsimd.memset(vT[m][:, 64::65], 1.0)
            nc.gpsimd.memset(evT[:, 64::65], 1.0)

            def blocks_ap(t, npart):
                full = t[:, :]
                return bass.AP(tensor=full.tensor, offset=full.offset,
                               ap=[[520, npart], [65, 8], [1, 64]])

            # ---------------- encoder kv (small, first) ----------------
            ek = [data.tile([128, ENC_L], bf, name=f"ek{p}") for p in range(4)]
            with nc.named_scope("ekv"):
                for half in range(2):
                    ps = big_ps.tile([128, 1024], f32, name="ek_ps", tag="st")
                    for i in range(2):
                        p = 2 * half + i
                        for k in range(6):
                            nc.tensor.matmul(
                                ps[:, 512 * i:512 * i + ENC_L],
                                wek[k][:, 128 * p:128 * (p + 1)], enct[k],
                                start=(k == 0), stop=(k == 5))
                    for i in range(2):
                        p = 2 * half + i
                        nc.vector.tensor_scalar_add(
                            out=ek[p], in0=ps[:, 512 * i:512 * i + ENC_L],
                            scalar1=bek[:, p:p + 1])
                ps = big_ps.tile([128, 1024], f32, name="ev_ps", tag="st")
                for k in range(6):
                    nc.tensor.matmul(ps[0:ENC_L, 0:512], enct[k], wev[k],
                                     start=(k == 0), stop=(k == 5))
                nc.vector.tensor_tensor(out=blocks_ap(evT, ENC_L),
                                        in0=ps[0:ENC_L, 0:512],
                                        in1=evbias_bc[0:ENC_L, :], op=OP.add)

            # ---------------- GroupNorm ----------------
            with nc.named_scope("gn"):
                stats = small.tile([128, 8], f32)
                for k in range(4):
                    nc.vector.reduce_sum(stats[:, k:k + 1], xt[k], axis=mybir.AxisListType.X)
                for k in range(4):
                    xsq = small.tile([128, 1024], f32, name="xsq", tag="xsq", bufs=2)
                    nc.scalar.activation(out=xsq, in_=xt[k], func=AF.Square,
                                         accum_out=stats[:, 4 + k:5 + k])
                stats_bf = small.tile([128, 8], bf)
                nc.vector.tensor_copy(out=stats_bf, in_=stats)
                gps = big_ps.tile([128, 1024], f32, name="gn_ps", tag="st")
                g8_ps = gps[0:8, 0:8]
                nc.tensor.matmul(g8_ps, emat, stats_bf, start=True, stop=True)
                musg = small.tile([8, 8], f32)   # cols 0:4 mean, 4:8 later rstd
                inv_n = 1.0 / (GS * L)
                nc.vector.tensor_scalar_mul(out=musg, in0=g8_ps, scalar1=inv_n)
                var8 = small.tile([8, 4], f32)
                nc.vector.tensor_mul(out=var8, in0=musg[:, 0:4], in1=musg[:, 0:4])
                nc.vector.tensor_sub(out=var8, in0=musg[:, 4:8], in1=var8)
                epst = small.tile([8, 1], f32)
                nc.vector.memset(epst, EPS)
                lnv = small.tile([8, 4], f32)
                nc.scalar.activation(out=lnv, in_=var8, func=AF.Ln, bias=epst, scale=1.0)
                nc.scalar.activation(out=musg[:, 4:8], in_=lnv, func=AF.Exp, scale=-0.5)
                musg_bf = small.tile([8, 8], bf)
                nc.vector.tensor_copy(out=musg_bf, in_=musg)
                exp_ps = gps[:, 512:520]
                nc.tensor.matmul(exp_ps, etmat, musg_bf, start=True, stop=True)
                aff_a = small.tile([128, 4], f32)
                nc.vector.tensor_mul(out=aff_a, in0=gnw, in1=exp_ps[:, 4:8])
                aff_b = small.tile([128, 4], f32)
                nc.vector.tensor_mul(out=aff_b, in0=exp_ps[:, 0:4], in1=aff_a)
                nc.vector.tensor_sub(out=aff_b, in0=gnb, in1=aff_b)
                hn = [data.tile([128, 1024], bf, name=f"hn{k}") for k in range(4)]
                for k in range(4):
                    nc.gpsimd.tensor_scalar(
                        out=hn[k], in0=xt[k], scalar1=aff_a[:, k:k + 1],
                        scalar2=aff_b[:, k:k + 1], op0=OP.mult, op1=OP.add)

            # ---------------- projections + attention, interleaved ----------------
            qk = [data.tile([128, 1024], bf, name=f"qk{m}") for m in range(8)]
            a_sb = [data.tile([128, 1024], bf, name=f"a_sb{p}") for p in range(4)]

            def emit_qk(m):
                ps = big_ps.tile([128, 1024], f32, name="qkv_ps", tag="st")
                for n in range(2):
                    for k in range(4):
                        nc.tensor.matmul(
                            ps[:, 512 * n:512 * (n + 1)],
                            wqk[k][:, 128 * m:128 * (m + 1)],
                            hn[k][:, 512 * n:512 * (n + 1)],
                            start=(k == 0), stop=(k == 3))
                nc.vector.tensor_scalar_add(out=qk[m], in0=ps, scalar1=bqk[:, m:m + 1])

            def emit_v2(m0):
                ps = big_ps.tile([128, 1024], f32, name="v_ps", tag="st")
                for i in range(2):
                    m = m0 + i
                    for k in range(4):
                        nc.tensor.matmul(
                            ps[:, 512 * i:512 * (i + 1)],
                            hn[k][:, 128 * m:128 * (m + 1)], wv[k],
                            start=(k == 0), stop=(k == 3))
                for i in range(2):
                    m = m0 + i
                    nc.vector.tensor_tensor(out=blocks_ap(vT[m], 128),
                                            in0=ps[:, 512 * i:512 * (i + 1)],
                                            in1=vbias_bc, op=OP.add)

            def emit_attention(p, interleave_v=False):
                qp, kp, ekp = qk[2 * p], qk[2 * p + 1], ek[p]
                avt = [[av_ps.tile([128, 512], f32, name=f"av{hh}{n}",
                                   tag=f"av{hh}{n}") for n in range(2)]
                       for hh in range(2)]
                nchunks = len(S_CHUNKS)
                for ci, (s0, sw) in enumerate(S_CHUNKS):
                    first, last = ci == 0, ci == nchunks - 1
                    pT = []
                    for hh in range(2):
                        pb = 64 * hh
                        st = big_ps.tile([128, 1024], f32, name="st", tag="st")
                        if first:
                            lhsT = ekp[pb:pb + 64, :]
                        else:
                            lhsT = kp[pb:pb + 64, s0 - ENC_L:s0 - ENC_L + sw]
                        for n in range(2):
                            nc.tensor.matmul(
                                st[0:sw, 512 * n:512 * (n + 1)],
                                lhsT, qp[pb:pb + 64, 512 * n:512 * (n + 1)],
                                start=True, stop=True)
                        pt = pts.tile([128, 1024], bf, name="pt", tag="pt")
                        nc.scalar.activation(out=pt[0:sw, :], in_=st[0:sw, :], func=AF.Exp)
                        pT.append(pt)
                    vsrc = evT if first else vT[ci - 1]
                    for hh in range(2):
                        h = 2 * p + hh
                        lhsTv = vsrc[0:sw, 65 * h:65 * h + 65]
                        for n in range(2):
                            nc.tensor.matmul(
                                avt[hh][n][0:65, :],
                                lhsTv, pT[hh][0:sw, 512 * n:512 * (n + 1)],
                                start=first, stop=last,
                                skip_group_check=True)
                    if interleave_v and ci in (1, 3, 5):
                        emit_v2(ci + 1)
                # normalization: A[c,t] / D[t], D in PSUM row 64 of each av
                # tile.  Round-trip raw D rows through DRAM to broadcast over
                # partitions, then a single fused tensor_tensor divide.
                dsb = small.tile([128, 512], f32, name="dsb", tag="dsb", bufs=2)
                for hh in range(2):
                    for n in range(2):
                        j = 2 * hh + n
                        nc.vector.reciprocal(out=dsb[32 * j:32 * j + 1, :],
                                             in_=avt[hh][n][64:65, :])
                ddr = ddr_pool.tile([4, 512], f32, name="ddr", tag="ddr")
                nc.sync.dma_start(out=ddr[0:4, :], in_=dsb[::32, :])
                dbc = [pts.tile([128, 512], f32, name=f"dbc{n}", tag=f"dbc{n}",
                                bufs=2) for n in range(2)]
                for n in range(2):
                    srcap = bass.AP(tensor=ddr.tensor, offset=512 * n,
                                    ap=[[1024, 2], [0, 64], [1, 512]])
                    deng = nc.sync if n == 0 else nc.gpsimd
                    deng.dma_start(out=dbc[n], in_=srcap)
                for n in range(2):
                    for hh in range(2):
                        nc.vector.tensor_tensor(
                            out=a_sb[p][64 * hh:64 * hh + 64, 512 * n:512 * (n + 1)],
                            in0=avt[hh][n][0:64, :],
                            in1=dbc[n][64 * hh:64 * hh + 64, :], op=OP.mult)

            with nc.named_scope("qkv"):
                emit_qk(0)
                emit_qk(1)
                emit_v2(0)
            with nc.named_scope("attn"):
                for p in range(4):
                    emit_attention(p, interleave_v=(p == 0))
                    if p < 3:
                        with nc.named_scope("qkv"):
                            emit_qk(2 * p + 2)
                            emit_qk(2 * p + 3)

            # ---------------- proj + residual ----------------
            with nc.named_scope("proj"):
                for m in range(4):
                    ps = big_ps.tile([128, 1024], f32, name="pj_ps", tag="st")
                    for n in range(2):
                        for k in range(4):
                            nc.tensor.matmul(
                                ps[:, 512 * n:512 * (n + 1)],
                                wp[k][:, 128 * m:128 * (m + 1)],
                                a_sb[k][:, 512 * n:512 * (n + 1)],
                                start=(k == 0), stop=(k == 3))
                    ot = data.tile([128, 1024], f32, name="ot", tag="ot", bufs=2)
                    nc.vector.scalar_tensor_tensor(
                        out=ot, in0=ps, scalar=bp[:, m:m + 1],
                        in1=xt[m], op0=OP.add, op1=OP.add)
                    deng = nc.sync if m % 2 == 0 else nc.gpsimd
                    deng.dma_start(out=out_d[128 * m:128 * (m + 1), :], in_=ot)
    nc.compile()
    return nc


def _host_prep(x, encoder_out, gn_w, gn_b, qkv_w, qkv_b, ekv_w, ekv_b, proj_w, proj_b):
    """Build per-core in_maps (weights replicated, batch sharded)."""
    x = np.asarray(x, np.float32).reshape(B, C, L)
    enc = np.asarray(encoder_out, np.float32)
    qkv_w = np.asarray(qkv_w, np.float32); qkv_b = np.asarray(qkv_b, np.float32)
    ekv_w = np.asarray(ekv_w, np.float32); ekv_b = np.asarray(ekv_b, np.float32)
    proj_w = np.asarray(proj_w, np.float32); proj_b = np.asarray(proj_b, np.float32)
    gn_w = np.asarray(gn_w, np.float32); gn_b = np.asarray(gn_b, np.float32)

    qk_order, v_order, ek_order, ev_order = [], [], [], []
    for p in range(4):
        for h in (2 * p, 2 * p + 1):
            qk_order += [192 * h + i for i in range(64)]
        for h in (2 * p, 2 * p + 1):
            qk_order += [192 * h + 64 + i for i in range(64)]
        for h in (2 * p, 2 * p + 1):
            ek_order += [128 * h + i for i in range(64)]
    for h in range(8):
        v_order += [192 * h + 128 + i for i in range(64)]
        ev_order += [128 * h + 64 + i for i in range(64)]

    wqk = (qkv_w[qk_order, :].T * SCALE).astype(BF16)
    bqk = (qkv_b[qk_order] * SCALE).astype(np.float32).reshape(8, 128).T.copy()
    wv = qkv_w[v_order, :].T.astype(BF16)
    bv = qkv_b[v_order].astype(BF16).reshape(1, 512)
    wek = (ekv_w[ek_order, :].T * SCALE).astype(BF16)
    bek = (ekv_b[ek_order] * SCALE).astype(np.float32).reshape(4, 128).T.copy()
    wev = ekv_w[ev_order, :].T.astype(BF16)
    bev = ekv_b[ev_order].astype(BF16).reshape(1, 512)
    wp = proj_w.T.astype(BF16)
    bp = proj_b.astype(np.float32).reshape(4, 128).T.copy()
    gnw4 = gn_w.reshape(4, 128).T.copy()
    gnb4 = gn_b.reshape(4, 128).T.copy()
    emat = np.zeros((128, 8), BF16)
    for pp in range(128):
        emat[pp, pp // 16] = 1
    etmat = np.ascontiguousarray(emat.T)

    shared = dict(
        wqk=np.ascontiguousarray(wqk), wv=np.ascontiguousarray(wv),
        wek=np.ascontiguousarray(wek), wev=np.ascontiguousarray(wev),
        wp=np.ascontiguousarray(wp),
        bqk=np.ascontiguousarray(bqk), bek=np.ascontiguousarray(bek),
        bv=bv, bev=bev, bp=np.ascontiguousarray(bp),
        gnw=np.ascontiguousarray(gnw4), gnb=np.ascontiguousarray(gnb4),
        emat=emat, etmat=etmat,
    )
    in_maps = []
    for b in range(B):
        m = dict(shared)
        m["x"] = np.ascontiguousarray(x[b])
        m["enc"] = np.ascontiguousarray(enc[b].astype(BF16))
        in_maps.append(m)
    return in_maps


_NC_CACHE = {}


def _get_nc():
    if "nc" not in _NC_CACHE:
        _NC_CACHE["nc"] = _build_bass()
    return _NC_CACHE["nc"]


def kernel(**inputs):
    from concourse.bass_utils import run_bass_kernel_spmd
    in_maps = _host_prep(**inputs)
    nc = _get_nc()
    res = run_bass_kernel_spmd(nc, in_maps, core_ids=list(range(N_CORES)))
    out = np.stack([res.results[b]["out"] for b in range(B)])
    return out.reshape(B, C, H, W).astype(np.float32)


# revision 8
# speedup vs baseline: 1.2317x; 1.0163x over previous
"""AttentionBlock Trainium2 kernel (nn_AttentionBlock dense_transformer).

Sharding: data-parallel over batch B=8 across 8 NeuronCores (1 image/core).
Per-core pipeline:
  - GroupNorm(32 groups) over x [512, 1024]; rstd via Newton rsqrt on DVE so
    the Activation engine only ever needs the Exp table (warmed once at t=0).
  - qkv / encoder_kv projections (bf16 matmuls, fp32 PSUM accumulate)
      q,k in [c, t] layout, v/ev transposed [s, 8x(64ch+ones)] layout.
  - attention: S^T = k^T q in [s, t] layout; softmax axis = partitions.
      Max-subtraction is skipped (logits are O(6) by construction: normalized
      activations x unit-variance weights, scale folded on host).
      exp on ScalarE; A = sum_s P v via matmuls whose lhsT (V) carries an
      extra ones-column per head, so PSUM row 64 accumulates the softmax
      denominator D for free (matmul cost depends only on the moving dim).
      1/D broadcast over partitions via a DRAM round-trip, applied in the
      fused PSUM->SBUF multiply.
  - proj + residual add; proj for two output blocks is partially accumulated
    during the last head-pair's normalization to hide the tail.
All matmul inputs bf16 (fp32 accumulation); end-to-end error vs fp32 reference
measured ~7e-4 relative.
"""

import numpy as np
import ml_dtypes

B, C, H, W = 8, 512, 32, 32
L = H * W                      # 1024
NH = 8
CH = C // NH                   # 64 per head
G = 32                         # groupnorm groups
GS = C // G                    # 16 channels per group
ENC_C, ENC_L = 768, 77
EPS = 1e-5
S_TOT = ENC_L + L              # 1101
SCALE = 1.0 / np.sqrt(np.sqrt(CH))
N_CORES = 8

# s-chunks of the key/value axis: enc block (77) then 8 x 128 self blocks
S_CHUNKS = [(0, ENC_L)] + [(ENC_L + 128 * i, 128) for i in range(8)]

BF16 = ml_dtypes.bfloat16


def _build_bass(debug=False):
    import concourse.bass as bass
    import concourse.mybir as mybir
    import concourse.tile as tile
    from concourse import bacc

    f32 = mybir.dt.float32
    bf = mybir.dt.bfloat16
    AF = mybir.ActivationFunctionType
    OP = mybir.AluOpType

    nc = bacc.Bacc()

    # ---- DRAM I/O ----
    x_d = nc.dram_tensor("x", [C, L], f32, kind="ExternalInput")
    enc_d = nc.dram_tensor("enc", [ENC_C, ENC_L], bf, kind="ExternalInput")
    wqk_d = nc.dram_tensor("wqk", [C, 1024], bf, kind="ExternalInput")
    wv_d = nc.dram_tensor("wv", [C, 512], bf, kind="ExternalInput")
    wek_d = nc.dram_tensor("wek", [ENC_C, 512], bf, kind="ExternalInput")
    wev_d = nc.dram_tensor("wev", [ENC_C, 512], bf, kind="ExternalInput")
    wp_d = nc.dram_tensor("wp", [C, C], bf, kind="ExternalInput")
    bqk_d = nc.dram_tensor("bqk", [128, 8], f32, kind="ExternalInput")
    bek_d = nc.dram_tensor("bek", [128, 4], f32, kind="ExternalInput")
    bv_d = nc.dram_tensor("bv", [1, 512], bf, kind="ExternalInput")
    bev_d = nc.dram_tensor("bev", [1, 512], bf, kind="ExternalInput")
    bp_d = nc.dram_tensor("bp", [128, 4], f32, kind="ExternalInput")
    gnw_d = nc.dram_tensor("gnw", [128, 4], f32, kind="ExternalInput")
    gnb_d = nc.dram_tensor("gnb", [128, 4], f32, kind="ExternalInput")
    emat_d = nc.dram_tensor("emat", [128, 8], bf, kind="ExternalInput")
    etmat_d = nc.dram_tensor("etmat", [8, 128], bf, kind="ExternalInput")
    out_d = nc.dram_tensor("out", [C, L], f32, kind="ExternalOutput")

    with tile.TileContext(nc) as tc:
        with tc.tile_pool(name="wpool", bufs=1) as wpool, \
             tc.tile_pool(name="data", bufs=1) as data, \
             tc.tile_pool(name="small", bufs=1) as small, \
             tc.tile_pool(name="pts", bufs=6) as pts, \
             tc.tile_pool(name="ddr", bufs=2, space="DRAM") as ddr_pool, \
             tc.tile_pool(name="big_ps", bufs=2, space="PSUM") as big_ps, \
             tc.tile_pool(name="av_ps", bufs=1, space="PSUM") as av_ps:

            # ---------------- loads ----------------
            # sync (SP) queue: x even blocks, enc, wek, small tensors.
            # gpsimd (Pool) queue: x odd blocks, wqk, wv, wev, wp.
            xt = [data.tile([128, 1024], f32, name=f"xt{k}") for k in range(4)]
            enct = [data.tile([128, ENC_L], bf, name=f"enct{k}") for k in range(6)]
            wek = [wpool.tile([128, 512], bf, name=f"wek{k}") for k in range(6)]
            wev = [wpool.tile([128, 512], bf, name=f"wev{k}") for k in range(6)]
            wqk = [wpool.tile([128, 1024], bf, name=f"wqk{k}") for k in range(4)]
            wv = [wpool.tile([128, 512], bf, name=f"wv{k}") for k in range(4)]
            wp = [wpool.tile([128, 512], bf, name=f"wp{k}") for k in range(4)]

            for k in range(4):
                eng = nc.sync if k % 2 == 0 else nc.gpsimd
                eng.dma_start(out=xt[k], in_=x_d[128 * k:128 * (k + 1), :])
            for k in range(6):
                nc.sync.dma_start(out=enct[k], in_=enc_d[128 * k:128 * (k + 1), :])
            for k in range(6):
                nc.sync.dma_start(out=wek[k], in_=wek_d[128 * k:128 * (k + 1), :])
            for k in range(4):
                nc.gpsimd.dma_start(out=wqk[k], in_=wqk_d[128 * k:128 * (k + 1), :])
            for k in range(4):
                nc.gpsimd.dma_start(out=wv[k], in_=wv_d[128 * k:128 * (k + 1), :])
            for k in range(6):
                nc.gpsimd.dma_start(out=wev[k], in_=wev_d[128 * k:128 * (k + 1), :])
            for k in range(4):
                nc.gpsimd.dma_start(out=wp[k], in_=wp_d[128 * k:128 * (k + 1), :])
            bqk = wpool.tile([128, 8], f32)
            nc.sync.dma_start(out=bqk, in_=bqk_d[:, :])
            bek = wpool.tile([128, 4], f32)
            nc.sync.dma_start(out=bek, in_=bek_d[:, :])
            bp = wpool.tile([128, 4], f32)
            nc.sync.dma_start(out=bp, in_=bp_d[:, :])
            gnw = wpool.tile([128, 4], f32)
            nc.sync.dma_start(out=gnw, in_=gnw_d[:, :])
            gnb = wpool.tile([128, 4], f32)
            nc.sync.dma_start(out=gnb, in_=gnb_d[:, :])
            emat = wpool.tile([128, 8], bf)
            nc.sync.dma_start(out=emat, in_=emat_d[:, :])
            etmat = wpool.tile([8, 128], bf)
            nc.sync.dma_start(out=etmat, in_=etmat_d[:, :])

            # Warm the Exp activation table once at t~0; every activation in
            # this kernel (Square, Copy, Exp) is served by the same set, so no
            # further 1.3us ACT_TABLE_LOADs occur.
            wtiny = small.tile([1, 8], f32)
            nc.vector.memset(wtiny, 1.0)
            wout = small.tile([1, 8], f32)
            nc.scalar.activation(out=wout, in_=wtiny, func=AF.Exp)

            # per-channel v / ev biases broadcast to all 128 partitions (DMA
            # partition-stride-0), added during the PSUM->SBUF v copies.
            vbias_bc = wpool.tile([128, 512], bf)
            evbias_bc = wpool.tile([128, 512], bf)
            for t, d in ((vbias_bc, bv_d), (evbias_bc, bev_d)):
                fd = d[:, :]
                nc.sync.dma_start(out=t, in_=bass.AP(
                    tensor=fd.tensor, offset=fd.offset, ap=[[0, 128], [1, 512]]))

            # V tiles: [s, 8 heads x (64 ch + ones col)].  The ones column per
            # head makes the AV matmul also accumulate the softmax denominator
            # into PSUM row 64 (lhsT columns = out partitions; cost unchanged).
            vT = [data.tile([128, 520], bf, name=f"vT{m}") for m in range(8)]
            evT = data.tile([ENC_L, 520], bf)
            for m in range(8):
                nc.gpsimd.memset(vT[m][:, 64::65], 1.0)
            nc.gpsimd.memset(evT[:, 64::65], 1.0)

            def blocks_ap(t, npart):
                full = t[:, :]
                return bass.AP(tensor=full.tensor, offset=full.offset,
                               ap=[[520, npart], [65, 8], [1, 64]])

            # ---------------- GroupNorm ----------------
            # GN's two tiny matmuls use the av_ps banks (idle until the first
            # attention AV matmul) so they don't occupy the big_ps ring.
            hn = [data.tile([128, 1024], bf, name=f"hn{k}") for k in range(4)]
            with nc.named_scope("gn"):
                stats = small.tile([128, 8], f32)
                for k in range(4):
                    nc.vector.reduce_sum(stats[:, k:k + 1], xt[k], axis=mybir.AxisListType.X)
                for k in range(4):
                    xsq = small.tile([128, 1024], f32, name="xsq", tag="xsq", bufs=2)
                    nc.scalar.activation(out=xsq, in_=xt[k], func=AF.Square,
                                         accum_out=stats[:, 4 + k:5 + k])
                stats_bf = small.tile([128, 8], bf)
                nc.vector.tensor_copy(out=stats_bf, in_=stats)
                gps = av_ps.tile([128, 512], f32, name="gn_g8", tag="av00")
                g8_ps = gps[0:8, 0:8]
                nc.tensor.matmul(g8_ps, emat, stats_bf, start=True, stop=True)
                musg = small.tile([8, 8], f32)   # cols 0:4 mean, 4:8 rstd
                inv_n = 1.0 / (GS * L)
                nc.vector.tensor_scalar_mul(out=musg, in0=g8_ps, scalar1=inv_n)
                var8 = small.tile([8, 4], f32)
                nc.vector.tensor_mul(out=var8, in0=musg[:, 0:4], in1=musg[:, 0:4])
                nc.vector.tensor_sub(out=var8, in0=musg[:, 4:8], in1=var8)
                nc.vector.tensor_scalar_add(out=var8, in0=var8, scalar1=EPS)
                # rstd = rsqrt(var+eps) by Newton iteration on DVE:
                # y0 = (1 + 1/v)/2, y <- y*(1.5 - 0.5*v*y^2) x3 (v ~ 1 here).
                yr = small.tile([8, 4], f32)
                nc.vector.reciprocal(out=yr, in_=var8)
                nc.vector.tensor_scalar(out=yr, in0=yr, scalar1=0.5, scalar2=0.5,
                                        op0=OP.mult, op1=OP.add)
                tn = small.tile([8, 4], f32)
                for _ in range(3):
                    nc.vector.tensor_mul(out=tn, in0=yr, in1=yr)
                    nc.vector.tensor_mul(out=tn, in0=tn, in1=var8)
                    nc.vector.tensor_scalar(out=tn, in0=tn, scalar1=-0.5,
                                            scalar2=1.5, op0=OP.mult, op1=OP.add)
                    nc.vector.tensor_mul(out=yr, in0=yr, in1=tn)
                nc.vector.tensor_copy(out=musg[:, 4:8], in_=yr)
                musg_bf = small.tile([8, 8], bf)
                nc.vector.tensor_copy(out=musg_bf, in_=musg)
                eps2 = av_ps.tile([128, 512], f32, name="gn_exp", tag="av01")
                exp_ps = eps2[:, 0:8]
                nc.tensor.matmul(exp_ps, etmat, musg_bf, start=True, stop=True)
                aff_a = small.tile([128, 4], f32)
                nc.vector.tensor_mul(out=aff_a, in0=gnw, in1=exp_ps[:, 4:8])
                aff_b = small.tile([128, 4], f32)
                nc.vector.tensor_mul(out=aff_b, in0=exp_ps[:, 0:4], in1=aff_a)
                nc.vector.tensor_sub(out=aff_b, in0=gnb, in1=aff_b)
                for k in range(4):
                    eng = nc.vector if k % 2 == 0 else nc.gpsimd
                    eng.tensor_scalar(
                        out=hn[k], in0=xt[k], scalar1=aff_a[:, k:k + 1],
                        scalar2=aff_b[:, k:k + 1], op0=OP.mult, op1=OP.add)

            # ---------------- encoder k (needed by attention chunk 0) --------
            ek = [data.tile([128, ENC_L], bf, name=f"ek{p}") for p in range(4)]
            with nc.named_scope("ekv"):
                for half in range(2):
                    ps = big_ps.tile([128, 1024], f32, name="ek_ps", tag="st")
                    for i in range(2):
                        p = 2 * half + i
                        for k in range(6):
                            nc.tensor.matmul(
                                ps[:, 512 * i:512 * i + ENC_L],
                                wek[k][:, 128 * p:128 * (p + 1)], enct[k],
                                start=(k == 0), stop=(k == 5))
                    for i in range(2):
                        p = 2 * half + i
                        nc.vector.tensor_scalar_add(
                            out=ek[p], in0=ps[:, 512 * i:512 * i + ENC_L],
                            scalar1=bek[:, p:p + 1])

            # ---------------- projections + attention, interleaved ----------
            qk = [data.tile([128, 1024], bf, name=f"qk{m}") for m in range(8)]
            a_sb = [data.tile([128, 1024], bf, name=f"a_sb{p}") for p in range(4)]

            def emit_qk(m):
                ps = big_ps.tile([128, 1024], f32, name="qkv_ps", tag="st")
                for n in range(2):
                    for k in range(4):
                        nc.tensor.matmul(
                            ps[:, 512 * n:512 * (n + 1)],
                            wqk[k][:, 128 * m:128 * (m + 1)],
                            hn[k][:, 512 * n:512 * (n + 1)],
                            start=(k == 0), stop=(k == 3))
                nc.vector.tensor_scalar_add(out=qk[m], in0=ps, scalar1=bqk[:, m:m + 1])

            def emit_ev():
                ps = big_ps.tile([128, 1024], f32, name="ev_ps", tag="st")
                for k in range(6):
                    nc.tensor.matmul(ps[0:ENC_L, 0:512], enct[k], wev[k],
                                     start=(k == 0), stop=(k == 5))
                nc.vector.tensor_tensor(out=blocks_ap(evT, ENC_L),
                                        in0=ps[0:ENC_L, 0:512],
                                        in1=evbias_bc[0:ENC_L, :], op=OP.add)

            def emit_v2(m0):
                ps = big_ps.tile([128, 1024], f32, name="v_ps", tag="st")
                for i in range(2):
                    m = m0 + i
                    for k in range(4):
                        nc.tensor.matmul(
                            ps[:, 512 * i:512 * (i + 1)],
                            hn[k][:, 128 * m:128 * (m + 1)], wv[k],
                            start=(k == 0), stop=(k == 3))
                for i in range(2):
                    m = m0 + i
                    nc.vector.tensor_tensor(out=blocks_ap(vT[m], 128),
                                            in0=ps[:, 512 * i:512 * (i + 1)],
                                            in1=vbias_bc, op=OP.add)

            pj_ps = {}

            def proj_partial(m):
                ps = big_ps.tile([128, 1024], f32, name="pj_ps", tag="st")
                pj_ps[m] = ps
                for n in range(2):
                    for k in range(3):
                        nc.tensor.matmul(
                            ps[:, 512 * n:512 * (n + 1)],
                            wp[k][:, 128 * m:128 * (m + 1)],
                            a_sb[k][:, 512 * n:512 * (n + 1)],
                            start=(k == 0), stop=False)

            def proj_finish(m):
                ps = pj_ps.pop(m, None)
                if ps is None:
                    ps = big_ps.tile([128, 1024], f32, name="pj_ps", tag="st")
                    for n in range(2):
                        for k in range(4):
                            nc.tensor.matmul(
                                ps[:, 512 * n:512 * (n + 1)],
                                wp[k][:, 128 * m:128 * (m + 1)],
                                a_sb[k][:, 512 * n:512 * (n + 1)],
                                start=(k == 0), stop=(k == 3))
                else:
                    for n in range(2):
                        nc.tensor.matmul(
                            ps[:, 512 * n:512 * (n + 1)],
                            wp[3][:, 128 * m:128 * (m + 1)],
                            a_sb[3][:, 512 * n:512 * (n + 1)],
                            start=False, stop=True)
                ot = data.tile([128, 1024], f32, name="ot", tag="ot", bufs=2)
                nc.vector.scalar_tensor_tensor(
                    out=ot, in0=ps, scalar=bp[:, m:m + 1],
                    in1=xt[m], op0=OP.add, op1=OP.add)
                deng = nc.sync if m % 2 == 0 else nc.gpsimd
                deng.dma_start(out=out_d[128 * m:128 * (m + 1), :], in_=ot)

            def emit_attention(p, interleave_v=False, tail_work=None):
                qp, kp, ekp = qk[2 * p], qk[2 * p + 1], ek[p]
                avt = [[av_ps.tile([128, 512], f32, name=f"av{hh}{n}",
                                   tag=f"av{hh}{n}") for n in range(2)]
                       for hh in range(2)]
                nchunks = len(S_CHUNKS)
                for ci, (s0, sw) in enumerate(S_CHUNKS):
                    first, last = ci == 0, ci == nchunks - 1
                    pT = []
                    for hh in range(2):
                        pb = 64 * hh
                        st = big_ps.tile([128, 1024], f32, name="st", tag="st")
                        if first:
                            lhsT = ekp[pb:pb + 64, :]
                        else:
                            lhsT = kp[pb:pb + 64, s0 - ENC_L:s0 - ENC_L + sw]
                        for n in range(2):
                            nc.tensor.matmul(
                                st[0:sw, 512 * n:512 * (n + 1)],
                                lhsT, qp[pb:pb + 64, 512 * n:512 * (n + 1)],
                                start=True, stop=True)
                        pt = pts.tile([128, 1024], bf, name="pt", tag="pt")
                        nc.scalar.activation(out=pt[0:sw, :], in_=st[0:sw, :], func=AF.Exp)
                        pT.append(pt)
                    vsrc = evT if first else vT[ci - 1]
                    for hh in range(2):
                        h = 2 * p + hh
                        lhsTv = vsrc[0:sw, 65 * h:65 * h + 65]
                        for n in range(2):
                            nc.tensor.matmul(
                                avt[hh][n][0:65, :],
                                lhsTv, pT[hh][0:sw, 512 * n:512 * (n + 1)],
                                start=first, stop=last,
                                skip_group_check=True)
                    if interleave_v and ci in (1, 3, 5):
                        emit_v2(ci + 1)
                if tail_work is not None:
                    tail_work()
                # normalization: A[c,t] / D[t], D in PSUM row 64 of each av
                # tile; 1/D staged through DRAM for a partition broadcast.
                dsb = small.tile([128, 512], f32, name="dsb", tag="dsb", bufs=2)
                for hh in range(2):
                    for n in range(2):
                        j = 2 * hh + n
                        nc.vector.reciprocal(out=dsb[32 * j:32 * j + 1, :],
                                             in_=avt[hh][n][64:65, :])
                ddr = ddr_pool.tile([4, 512], f32, name="ddr", tag="ddr")
                nc.sync.dma_start(out=ddr[0:4, :], in_=dsb[::32, :])
                dbc = [pts.tile([128, 512], f32, name=f"dbc{n}", tag=f"dbc{n}",
                                bufs=2) for n in range(2)]
                for n in range(2):
                    srcap = bass.AP(tensor=ddr.tensor, offset=512 * n,
                                    ap=[[1024, 2], [0, 64], [1, 512]])
                    nc.sync.dma_start(out=dbc[n], in_=srcap)
                for n in range(2):
                    for hh in range(2):
                        nc.vector.tensor_tensor(
                            out=a_sb[p][64 * hh:64 * hh + 64, 512 * n:512 * (n + 1)],
                            in0=avt[hh][n][0:64, :],
                            in1=dbc[n][64 * hh:64 * hh + 64, :], op=OP.mult)

            with nc.named_scope("qkv"):
                emit_qk(0)
                emit_qk(1)
                emit_ev()
                emit_v2(0)
            with nc.named_scope("attn"):
                for p in range(4):
                    tail = None
                    if p == 3:
                        tail = lambda: (proj_partial(0), proj_partial(1))
                    emit_attention(p, interleave_v=(p == 0), tail_work=tail)
                    if p < 3:
                        with nc.named_scope("qkv"):
                            emit_qk(2 * p + 2)
                            emit_qk(2 * p + 3)

            # ---------------- proj + residual ----------------
            with nc.named_scope("proj"):
                for m in range(4):
                    proj_finish(m)
    nc.compile()
    return nc


def _host_prep(x, encoder_out, gn_w, gn_b, qkv_w, qkv_b, ekv_w, ekv_b, proj_w, proj_b):
    """Build per-core in_maps (weights replicated, batch sharded)."""
    x = np.asarray(x, np.float32).reshape(B, C, L)
    enc = np.asarray(encoder_out, np.float32)
    qkv_w = np.asarray(qkv_w, np.float32); qkv_b = np.asarray(qkv_b, np.float32)
    ekv_w = np.asarray(ekv_w, np.float32); ekv_b = np.asarray(ekv_b, np.float32)
    proj_w = np.asarray(proj_w, np.float32); proj_b = np.asarray(proj_b, np.float32)
    gn_w = np.asarray(gn_w, np.float32); gn_b = np.asarray(gn_b, np.float32)

    qk_order, v_order, ek_order, ev_order = [], [], [], []
    for p in range(4):
        for h in (2 * p, 2 * p + 1):
            qk_order += [192 * h + i for i in range(64)]
        for h in (2 * p, 2 * p + 1):
            qk_order += [192 * h + 64 + i for i in range(64)]
        for h in (2 * p, 2 * p + 1):
            ek_order += [128 * h + i for i in range(64)]
    for h in range(8):
        v_order += [192 * h + 128 + i for i in range(64)]
        ev_order += [128 * h + 64 + i for i in range(64)]

    wqk = (qkv_w[qk_order, :].T * SCALE).astype(BF16)
    bqk = (qkv_b[qk_order] * SCALE).astype(np.float32).reshape(8, 128).T.copy()
    wv = qkv_w[v_order, :].T.astype(BF16)
    bv = qkv_b[v_order].astype(BF16).reshape(1, 512)
    wek = (ekv_w[ek_order, :].T * SCALE).astype(BF16)
    bek = (ekv_b[ek_order] * SCALE).astype(np.float32).reshape(4, 128).T.copy()
    wev = ekv_w[ev_order, :].T.astype(BF16)
    bev = ekv_b[ev_order].astype(BF16).reshape(1, 512)
    wp = proj_w.T.astype(BF16)
    bp = proj_b.astype(np.float32).reshape(4, 128).T.copy()
    gnw4 = gn_w.reshape(4, 128).T.copy()
    gnb4 = gn_b.reshape(4, 128).T.copy()
    emat = np.zeros((128, 8), BF16)
    for pp in range(128):
        emat[pp, pp // 16] = 1
    etmat = np.ascontiguousarray(emat.T)

    shared = dict(
        wqk=np.ascontiguousarray(wqk), wv=np.ascontiguousarray(wv),
        wek=np.ascontiguousarray(wek), wev=np.ascontiguousarray(wev),
        wp=np.ascontiguousarray(wp),
        bqk=np.ascontiguousarray(bqk), bek=np.ascontiguousarray(bek),
        bv=bv, bev=bev, bp=np.ascontiguousarray(bp),
        gnw=np.ascontiguousarray(gnw4), gnb=np.ascontiguousarray(gnb4),
        emat=emat, etmat=etmat,
    )
    in_maps = []
    for b in range(B):
        m = dict(shared)
        m["x"] = np.ascontiguousarray(x[b])
        m["enc"] = np.ascontiguousarray(enc[b].astype(BF16))
        in_maps.append(m)
    return in_maps


_NC_CACHE = {}


def _get_nc():
    if "nc" not in _NC_CACHE:
        _NC_CACHE["nc"] = _build_bass()
    return _NC_CACHE["nc"]


def kernel(**inputs):
    from concourse.bass_utils import run_bass_kernel_spmd
    in_maps = _host_prep(**inputs)
    nc = _get_nc()
    res = run_bass_kernel_spmd(nc, in_maps, core_ids=list(range(N_CORES)))
    out = np.stack([res.results[b]["out"] for b in range(B)])
    return out.reshape(B, C, H, W).astype(np.float32)
